# revision 37
# baseline (speedup 1.0000x reference)
"""BSTGCNet fused Trainium2 kernel (8 NeuronCores, batch-parallel), v2.

Math (per batch element b, handled entirely on core b):
  For each t in 0..11, each GAT g in {s,n,d}:
    x = X[b,t]                                [N=512, F=2]
    Wh = x @ W_g [512,64];  u = x@c1_g, v = x@c2_g (c = W_g @ a_g)
    P[i,j] = adj_ij * exp(leaky(u_i + v_j)); f_g = elu((P@Wh)_i / sum_j P_ij)
  spatial = relu(Wf^T [f_s; f_n + f_d] + bf)       [64, 512]
  GRU over t on [spatial; x^T], head -> out [12, 512]

Implementation notes (all [feature/source-node partition, node free] layout):
  * exp(leaky(s)) = e^{0.2 u_i} * max(e^{0.8 u_i} e^{v_j}, e^{0.2 v_j}); the
    e^{0.2 u_i} column factor cancels in the softmax, so the attention tile is
    wb = (E_bcast * B_j) max b_j   -- one dual-scalar tensor_scalar (4x mode)
    pm = min(wb, adj * 1e30)       -- one tensor_tensor min (mask exactness:
                                      adj in {0,1}, wb in (0, ~4))
  * E row is partition-broadcast on GPSIMD (no DRAM bounce, no DMA).
  * The attention matmul lhsT is [Wh | ones*64] so raw2[64:128,:] comes out
    as the softmax denominator replicated across 64 partitions; the division
    is then a single tensor_tensor divide, no reciprocal / ln / broadcast.
  * elu is distributed through the spatial matmul: Wf^T elu(y) =
    Wf^T relu(y) + Wf^T min(e^y,1) - Wf^T 1, with the constant folded into
    the relu bias host-side.  6 accumulating matmuls per t.
  * The GRU is interleaved with stage A (it fills engine slack).  Only
    Exp/Tanh/Relu/Identity activations are used -- one ACT table set
    (exp_and_others), zero table reloads.  sigmoid(x) = 0.5 + 0.5 tanh(x/2)
    reconstructed with a dual-scalar tensor_scalar; all GRU biases are folded
    into the matmuls via ones-rows appended to xtT and hT.
"""

import numpy as np

B, T, N, FIN, H, P = 8, 12, 512, 2, 64, 12
NCORES = 8
NJT = 4  # 512 nodes / 128 partitions
BIG = 1e30

_NC_CACHE = {}
_LAST_RESULT = None


def _build():
    import concourse.bass as bass
    import concourse.bacc as bacc
    import concourse.mybir as mybir
    import concourse.tile as tile

    F32 = mybir.dt.float32
    BF = mybir.dt.bfloat16
    AF = mybir.ActivationFunctionType
    OP = mybir.AluOpType

    nc = bacc.Bacc("TRN2", target_bir_lowering=False)

    # ---- DRAM I/O (all host-prepacked) ----
    d_xt = nc.dram_tensor("xt", [FIN, T * N], BF, kind="ExternalInput")
    d_adjH = nc.dram_tensor("adjH", [128, 12 * N], BF, kind="ExternalInput")
    d_cu3 = nc.dram_tensor("cu3", [FIN, 65], BF, kind="ExternalInput")
    d_waug = nc.dram_tensor("waug", [FIN, 3 * 65], BF, kind="ExternalInput")
    d_wihrz = nc.dram_tensor("wihrz", [67, 2 * H], BF, kind="ExternalInput")
    d_wihn = nc.dram_tensor("wihn", [67, H], BF, kind="ExternalInput")
    d_whhrz = nc.dram_tensor("whhrz", [H, 2 * H], BF, kind="ExternalInput")
    d_whhn = nc.dram_tensor("whhn", [H + 1, H], BF, kind="ExternalInput")
    # whhn is pre-scaled by 0.5 host-side (the r = 0.5+0.5*Tr fold)
    d_wf = nc.dram_tensor("wf", [2 * H, H], BF, kind="ExternalInput")
    d_bf2 = nc.dram_tensor("bf2", [H, 1], F32, kind="ExternalInput")
    d_w1 = nc.dram_tensor("w1", [H, H // 2], BF, kind="ExternalInput")
    d_b1 = nc.dram_tensor("b1", [H // 2, 1], F32, kind="ExternalInput")
    d_w2 = nc.dram_tensor("w2", [H // 2, P], BF, kind="ExternalInput")
    d_b2 = nc.dram_tensor("b2", [P, 1], F32, kind="ExternalInput")
    d_out = nc.dram_tensor("out", [P, N], F32, kind="ExternalOutput")

    with tile.TileContext(nc) as tc:
        with tc.tile_pool(name="const", bufs=1) as const, \
             tc.tile_pool(name="xtT", bufs=1) as xtT_pool, \
             tc.tile_pool(name="e3p", bufs=3) as e3_pool, \
             tc.tile_pool(name="ebp", bufs=6) as eb_pool, \
             tc.tile_pool(name="colp", bufs=6) as col_pool, \
             tc.tile_pool(name="wbp", bufs=4) as wb_pool, \
             tc.tile_pool(name="pmp", bufs=4) as pm_pool, \
             tc.tile_pool(name="yp", bufs=4) as y_pool, \
             tc.tile_pool(name="gru", bufs=4) as gru_pool:

            # ---- constants / params (adjacency split per-g, loaded after
            # the small hot tensors so the first atoms start sooner) ----
            xt_sb = const.tile([FIN, T * N], BF)
            nc.sync.dma_start(out=xt_sb[:], in_=d_xt[:, :])
            cu3 = const.tile([FIN, 65], BF)
            nc.sync.dma_start(out=cu3[:], in_=d_cu3[:, :])
            waug = const.tile([FIN, 3 * 65], BF)
            nc.sync.dma_start(out=waug[:], in_=d_waug[:, :])
            adjH = const.tile([128, 12 * N], BF)
            for g_ in range(3):
                nc.sync.dma_start(out=adjH[:, 4 * g_ * N:(4 * g_ + 4) * N],
                                  in_=d_adjH[:, 4 * g_ * N:(4 * g_ + 4) * N])
            wihrz = const.tile([67, 2 * H], BF)
            nc.sync.dma_start(out=wihrz[:], in_=d_wihrz[:, :])
            wihn = const.tile([67, H], BF)
            nc.sync.dma_start(out=wihn[:], in_=d_wihn[:, :])
            whhrz = const.tile([H, 2 * H], BF)
            nc.sync.dma_start(out=whhrz[:], in_=d_whhrz[:, :])
            whhn05 = const.tile([H + 1, H], BF)
            nc.sync.dma_start(out=whhn05[:], in_=d_whhn[:, :])
            ones1 = const.tile([65, 128], BF)
            nc.gpsimd.memset(ones1[:], 1.0)
            wfa = const.tile([H, H], BF)
            nc.sync.dma_start(out=wfa[:], in_=d_wf[0:H, :])
            wfb = const.tile([H, H], BF)
            nc.sync.dma_start(out=wfb[:], in_=d_wf[H:2 * H, :])
            bf2 = const.tile([H, 1], F32)
            nc.sync.dma_start(out=bf2[:], in_=d_bf2[:, :])
            w1 = const.tile([H, H // 2], BF)
            nc.sync.dma_start(out=w1[:], in_=d_w1[:, :])
            b1 = const.tile([H // 2, 1], F32)
            nc.sync.dma_start(out=b1[:], in_=d_b1[:, :])
            w2 = const.tile([H // 2, P], BF)
            nc.sync.dma_start(out=w2[:], in_=d_w2[:, :])
            b2 = const.tile([P, 1], F32)
            nc.sync.dma_start(out=b2[:], in_=d_b2[:, :])

            # [spatial(0:64); x(64:66); ones(66)] for all t -- GRU inputs
            xtT = xtT_pool.tile([67, T * N], BF, tag="xtT")
            nc.gpsimd.memset(xtT[H:H + FIN + 1, :], 1.0)
            nc.sync.dma_start(out=xtT[H:H + FIN, :], in_=d_xt[:, :])

            # attention-matmul lhsT slots: [Wh | ones*64] per jt block
            whs_slots = []
            for s in range(2):
                w = const.tile([128, NJT * 128], BF, tag=f"whs{s}")
                wv = w[:].rearrange("p (j c) -> p j c", j=NJT)
                nc.gpsimd.memset(wv[:, :, 0:H], 1.0)
                whs_slots.append(w)

            # GRU hidden state double buffer, row 64 == 1 (bhn bias row)
            h_slots = []
            for s in range(2):
                h = const.tile([H + 1, N], BF, tag=f"h{s}")
                nc.gpsimd.memset(h[0:H, :], 0.0)
                nc.gpsimd.memset(h[H:H + 1, :], 1.0)
                h_slots.append(h)

            _cm_u = tc.tile_pool(name="ps_u", bufs=1, space="PSUM")
            _cm_wh = tc.tile_pool(name="ps_wh", bufs=2, space="PSUM")
            _cm_raw = tc.tile_pool(name="ps_raw", bufs=2, space="PSUM")
            _cm_sp = tc.tile_pool(name="ps_sp", bufs=1, space="PSUM")
            _cm_rzxn = tc.tile_pool(name="ps_rzxn", bufs=1, space="PSUM")
            _cm_hb = tc.tile_pool(name="ps_hb", bufs=1, space="PSUM")
            ps_u_pool = _cm_u.__enter__()
            ps_wh_pool = _cm_wh.__enter__()
            ps_raw_pool = _cm_raw.__enter__()
            ps_sp_pool = _cm_sp.__enter__()
            ps_rzxn_pool = _cm_rzxn.__enter__()
            ps_hb_pool = _cm_hb.__enter__()
            ps_eb_pool = ps_u_pool

            def emit_gru(t):
                tsl = slice(t * N, (t + 1) * N)
                h_prev = h_slots[t % 2]
                h_new = h_slots[(t + 1) % 2]
                ps_rz = ps_rzxn_pool.tile([2 * H, N], F32, tag="rzxn")
                nc.tensor.matmul(ps_rz[:], wihrz[:], xtT[:, tsl],
                                 start=True, stop=False)
                nc.tensor.matmul(ps_rz[:], whhrz[:], h_prev[0:H, :],
                                 start=False, stop=True)
                trz = gru_pool.tile([2 * H, N], BF, tag="trz")
                nc.scalar.activation(trz[:], ps_rz[:], AF.Tanh, scale=0.5)
                z = gru_pool.tile([H, N], BF, tag="z")
                nc.vector.tensor_scalar(z[:], trz[H:2 * H, :], 0.5, 0.5,
                                        OP.mult, OP.add)
                # n-arg = xn + bin + 0.5*hb + 0.5*Tr*hb  (r = 0.5 + 0.5*Tr)
                ps_hb = ps_hb_pool.tile([H, N], F32, tag="pshb")
                nc.tensor.matmul(ps_hb[:], whhn05[:], h_prev[:],
                                 start=True, stop=True)
                ps_xn = ps_rzxn_pool.tile([H, N], F32, tag="rzxn")
                nc.tensor.matmul(ps_xn[:], wihn[:], xtT[:, tsl],
                                 start=True, stop=False)
                nc.tensor.matmul(ps_xn[:], whhn05[:], h_prev[:],
                                 start=False, stop=True)
                hbs = gru_pool.tile([H, N], BF, tag="hbs")
                nc.scalar.copy(hbs[:], ps_hb[:])
                q = gru_pool.tile([H, N], BF, tag="q")
                nc.vector.tensor_mul(q[:], trz[0:H, :], hbs[:])
                xns = gru_pool.tile([H, N], BF, tag="xns")
                nc.scalar.copy(xns[:], ps_xn[:])
                s = gru_pool.tile([H, N], BF, tag="s")
                nc.vector.tensor_add(s[:], q[:], xns[:])
                n_ = gru_pool.tile([H, N], BF, tag="n")
                nc.scalar.activation(n_[:], s[:], AF.Tanh)
                # h' = n + z*(h - n)
                d_ = gru_pool.tile([H, N], BF, tag="d")
                nc.vector.tensor_sub(d_[:], h_prev[0:H, :], n_[:])
                zd = gru_pool.tile([H, N], BF, tag="zd")
                nc.vector.tensor_mul(zd[:], z[:], d_[:])
                nc.vector.tensor_add(h_new[0:H, :], n_[:], zd[:])

            for t in range(T):
                tsl = slice(t * N, (t + 1) * N)
                # u rows for the 3 gats (scaled by 0.8)
                ps_u = ps_u_pool.tile([65, N], F32, tag="psu")
                nc.tensor.matmul(ps_u[:], cu3[:], xt_sb[:, tsl],
                                 start=True, stop=True)
                u3 = e3_pool.tile([65, N], BF, tag="u3")
                nc.scalar.copy(u3[:], ps_u[:])

                y3 = y_pool.tile([H, 3 * N], BF, tag="y3")
                for g in range(3):
                    atom = 3 * t + g
                    # Wh / v for this gat: [128, 4*65] psum
                    ps_wh = ps_wh_pool.tile([128, NJT * 65], F32, tag="pswh")
                    pswh_v = ps_wh[:].rearrange("p (j c) -> p j c", j=NJT)
                    for jt in range(NJT):
                        nc.tensor.matmul(
                            ps_wh[:, jt * 65:(jt + 1) * 65],
                            xt_sb[:, t * N + jt * 128: t * N + (jt + 1) * 128],
                            waug[:, g * 65:(g + 1) * 65],
                            start=True, stop=True)
                    # B = e^v, b = e^{0.2 v} columns
                    bcol = col_pool.tile([128, NJT], F32, tag="bcol")
                    nc.scalar.activation(bcol[:], pswh_v[:, :, 64:65], AF.Exp)
                    bccol = col_pool.tile([128, NJT], F32, tag="bccol")
                    nc.scalar.activation(bccol[:], pswh_v[:, :, 64:65],
                                         AF.Exp, scale=0.2)
                    # whs slot: copy Wh into cols 0:64 of each 128-block
                    whs = whs_slots[atom % 2]
                    whs_v = whs[:].rearrange("p (j c) -> p j c", j=NJT)
                    nc.scalar.copy(whs_v[:, :, H:128], pswh_v[:, :, 0:H])

                    # E broadcast: PE ones-matmul of the 0.8u row, then
                    # the mandatory exp doubles as the PSUM->SBUF move
                    ps_eb = ps_eb_pool.tile([128, N], F32, tag="psu")
                    nc.tensor.matmul(ps_eb[:],
                                     ones1[32 * g:32 * g + 1, :],
                                     u3[32 * g:32 * g + 1, :],
                                     start=True, stop=True)
                    E_b = eb_pool.tile([128, N], BF, tag="Eb")
                    nc.scalar.activation(E_b[:], ps_eb[:], AF.Exp)

                    # attention tiles: wb = (E*B) max b ; pm = min(wb, adj*BIG)
                    wb4 = wb_pool.tile([128, NJT * N], BF, tag="wb4")
                    wb4_v = wb4[:].rearrange("p (j i) -> p j i", j=NJT)
                    wb_eng = nc.gpsimd if g >= 1 else nc.vector
                    for jt in range(NJT):
                        wb_eng.tensor_scalar(
                            wb4_v[:, jt, :], E_b[:], bcol[:, jt:jt + 1],
                            bccol[:, jt:jt + 1], OP.mult, OP.max)
                    pm4 = pm_pool.tile([128, NJT * N], BF, tag="pm4")
                    nc.vector.tensor_mul(
                        pm4[:], wb4[:],
                        adjH[:, 4 * g * N:(4 * g + 4) * N])

                    # raw2: rows 0:64 = P @ Wh (transposed), 64:128 = denom
                    raw2 = ps_raw_pool.tile([128, N], F32, tag="raw2")
                    for jt in range(NJT):
                        nc.tensor.matmul(raw2[:],
                                         whs[:, jt * 128:(jt + 1) * 128],
                                         pm4[:, jt * N:(jt + 1) * N],
                                         start=(jt == 0), stop=(jt == 3))
                    # raw2 rows 0:64 = den (replicated), 64:128 = num;
                    # rcp must read base partition 0 (custom-DVE constraint)
                    rcp = eb_pool.tile([H, N], F32, tag="rcp")
                    nc.vector.reciprocal_approx_fast(out=rcp[:],
                                                     in_=raw2[0:H, :])
                    nc.vector.tensor_mul(y3[:, g * N:(g + 1) * N],
                                         raw2[H:128, :], rcp[:])

                # elu split: r1 = relu(y), e1 = min(e^y, 1); -1 folded in bf2
                ey3 = y_pool.tile([H, 3 * N], BF, tag="ey3")
                nc.scalar.activation(ey3[:], y3[:], AF.Exp)
                r13 = y_pool.tile([H, 3 * N], BF, tag="r13")
                nc.vector.tensor_scalar(r13[:], y3[:], 0.0, None, OP.max)
                e13 = y_pool.tile([H, 3 * N], BF, tag="e13")
                nc.vector.tensor_scalar(e13[:], ey3[:], 1.0, None, OP.min)

                # spatial = relu(Wf^T [elu_s; elu_n + elu_d] + bf2)
                ps_sp = ps_sp_pool.tile([H, N], F32, tag="sp")
                parts = [(wfa, r13[:, 0:N]), (wfa, e13[:, 0:N]),
                         (wfb, r13[:, N:2 * N]), (wfb, e13[:, N:2 * N]),
                         (wfb, r13[:, 2 * N:3 * N]), (wfb, e13[:, 2 * N:3 * N])]
                for i, (lhs, rhs) in enumerate(parts):
                    nc.tensor.matmul(ps_sp[:], lhs[:], rhs,
                                     start=(i == 0), stop=(i == 5))
                nc.scalar.activation(xtT[0:H, tsl], ps_sp[:], AF.Relu,
                                     bias=bf2[:])
                emit_gru(t)

            # ---- head ----
            h_fin = h_slots[T % 2]
            ps_z1 = ps_rzxn_pool.tile([H // 2, N], F32, tag="rzxn")
            nc.tensor.matmul(ps_z1[:], w1[:], h_fin[0:H, :],
                             start=True, stop=True)
            z1 = gru_pool.tile([H // 2, N], BF, tag="z1")
            nc.scalar.activation(z1[:], ps_z1[:], AF.Relu, bias=b1[:])
            ps_o = ps_rzxn_pool.tile([P, N], F32, tag="rzxn")
            nc.tensor.matmul(ps_o[:], w2[:], z1[:], start=True, stop=True)
            osb = gru_pool.tile([P, N], F32, tag="osb")
            nc.scalar.activation(osb[:], ps_o[:], AF.Identity, bias=b2[:])
            nc.sync.dma_start(out=d_out[:, :], in_=osb[:])

            _cm_hb.__exit__(None, None, None)
            _cm_rzxn.__exit__(None, None, None)
            _cm_sp.__exit__(None, None, None)
            _cm_raw.__exit__(None, None, None)
            _cm_wh.__exit__(None, None, None)
            _cm_u.__exit__(None, None, None)

    nc.finalize()
    return nc


def _get_nc():
    if "nc" not in _NC_CACHE:
        _NC_CACHE["nc"] = _build()
    return _NC_CACHE["nc"]


def kernel(X, G_s, G_n, G_d, Wg, a1g, a2g, Wn, a1n, a2n, Wd, a1d, a2d,
           Wf, bf, W_ih, W_hh, b_ih, b_hh, W1, b1, W2, b2):
    import ml_dtypes
    from concourse.bass_utils import run_bass_kernel_spmd

    bf16 = ml_dtypes.bfloat16
    f32 = np.float32
    X = np.asarray(X, f32)

    # adjH[p, (4g+jt)*512 + i] = G_g[i, jt*128+p] * BIG
    adjH = np.zeros((128, 12 * N), f32)
    for g, G in enumerate((G_s, G_n, G_d)):
        GT = (np.asarray(G, f32) > 0).astype(f32)  # [i, j], {0,1}
        for jt in range(NJT):
            blk = GT[:, jt * 128:(jt + 1) * 128].T  # [128p, 512i]
            adjH[:, (4 * g + jt) * N:(4 * g + jt + 1) * N] = blk

    XT = np.ascontiguousarray(X.transpose(0, 3, 1, 2)).reshape(B, FIN, T * N)

    waug_l, cu_l = [], []
    for W, a1, a2 in ((Wg, a1g, a2g), (Wn, a1n, a2n), (Wd, a1d, a2d)):
        W = np.asarray(W, f32)
        c1 = W @ np.asarray(a1, f32)                       # [2,1]
        c2 = W @ np.asarray(a2, f32)                       # [2,1]
        waug_l.append(np.concatenate([W, c2], axis=1))     # [2,65]
        cu_l.append(0.8 * c1)                              # [2,1]
    waug = np.ascontiguousarray(np.concatenate(waug_l, axis=1), f32)  # [2,195]
    cu3 = np.zeros((FIN, 65), f32)
    for g in range(3):
        cu3[:, 32 * g:32 * g + 1] = cu_l[g]

    W_ih = np.asarray(W_ih, f32)
    W_hh = np.asarray(W_hh, f32)
    b_ih = np.asarray(b_ih, f32)
    b_hh = np.asarray(b_hh, f32)
    wihT = np.ascontiguousarray(W_ih.T)          # [66, 192]
    whhT = np.ascontiguousarray(W_hh.T)          # [64, 192]
    wihrz = np.concatenate([wihT[:, :2 * H],
                            (b_ih + b_hh)[:2 * H].reshape(1, -1)], axis=0)
    wihn = np.concatenate([wihT[:, 2 * H:],
                           b_ih[2 * H:].reshape(1, -1)], axis=0)
    whhn = 0.5 * np.concatenate([whhT[:, 2 * H:],
                                 b_hh[2 * H:].reshape(1, -1)], axis=0)

    Wf = np.asarray(Wf, f32)
    # elu = r1 + e1 - 1 per cat row; fs contributes -1*colsum(Wfa), and
    # fnd = elu_n + elu_d contributes -2*colsum(Wfb)
    bf2 = (np.asarray(bf, f32) - Wf[:H].sum(axis=0)
           - 2.0 * Wf[H:].sum(axis=0))

    common = dict(
        adjH=adjH, cu3=cu3, waug=waug,
        wihrz=np.ascontiguousarray(wihrz),
        wihn=np.ascontiguousarray(wihn),
        whhrz=np.ascontiguousarray(whhT[:, :2 * H]),
        whhn=np.ascontiguousarray(whhn),
        wf=Wf,
        bf2=np.ascontiguousarray(bf2.reshape(-1, 1), f32),
        w1=np.asarray(W1, f32),
        b1=np.ascontiguousarray(np.asarray(b1, f32).reshape(-1, 1)),
        w2=np.asarray(W2, f32),
        b2=np.ascontiguousarray(np.asarray(b2, f32).reshape(-1, 1)),
    )
    bf_keys = ("adjH", "cu3", "waug", "wihrz", "wihn", "whhrz", "whhn",
               "wf", "w1", "w2")
    common = {k: (v.astype(bf16) if k in bf_keys else v)
              for k, v in common.items()}
    in_maps = [dict(common, xt=np.ascontiguousarray(XT[b]).astype(bf16))
               for b in range(B)]

    nc = _get_nc()
    res = run_bass_kernel_spmd(nc, in_maps, core_ids=list(range(NCORES)))
    global _LAST_RESULT
    _LAST_RESULT = res
    out = np.stack([res.results[b]["out"] for b in range(B)])  # [B, P, N]
    return out.astype(f32)
